# revision 41
# baseline (speedup 1.0000x reference)
"""Trainium2 Bass kernel for nn_CalculateSLayer (GNN message passing).

Computes, for adj (N, N, 2) f32 and s (N, D) f32:
    a     = adj.sum(axis=2)                  # (N, N)
    s_in  = a.T @ s                          # (N, D)
    s_out = a @ s                            # (N, D)
returns (s_in, s_out) — matching the reference's output tuple.

Distribution: adjacency is sharded row-wise across 8 NeuronCores.  Core c
owns rows I_c = [c*512, (c+1)*512).  From its (512, 4096, 2) block it
computes on-device:
  * a partial s_in^T (D, N)   = (s[I_c]).T @ a[I_c]          (contract i)
  * its exact  s_out^T (D,512)= s.T-weighted a[I_c].T        (contract j)
The host then sums the 8 s_in partials and concatenates the s_out blocks.

Per-core dataflow (all engines pipelined under Tile):
  DMA: stream (128, 512j, 2k) f32 chunks of the raw block (4 KB/partition
       contiguous lines — peak-efficiency HBM reads).
  DVE: channel-reduce  a_chunk[:, j] = raw[:, j, 0] + raw[:, j, 1].
  PE : s_in matmul   psum_sin(70,512)  += s_own[it].T @ a_chunk   (f32r moving)
       transposes    psum_T(128,512)[.,it*128:] = a_tile.T        (via identity)
       s_out matmul  psum_out(70,512) += s_all[jt].T @ aT_chunk   (f32r moving)
  ACT: evacuate transposed tiles PSUM->SBUF.
  DMA: psum_sin -> s_inT DRAM per chunk; psum_out -> s_outT DRAM at end.

f32r (float32r) runs the PE at 1 cycle/row for moving free-dim >= 256 (vs 4
cycles/row for plain f32); transposes use the exact permutation path.
"""

import numpy as np

import concourse.bass as bass
from concourse import bacc
import concourse.mybir as mybir
import concourse.tile as tile
from concourse import bass_utils

N = 4096          # nodes
D = 70            # embedding dim
NCORES = 8
RB = N // NCORES  # 512 rows per core
P = 128           # partitions
IT = RB // P      # 4 i-tiles per core
WJ = 512          # j-chunk width
JC = N // WJ      # 8 j-chunks
JT = WJ // P      # 4 transpose subtiles per chunk
NJT = N // P      # 32 s_all subtiles

F32 = mybir.dt.float32
F32R = mybir.dt.float32r

# Set by the test harness to capture a profile; the grading path leaves these
# untouched.
TRACE = False
TRACE_KWARGS = {}
LAST_RESULT = None


def _emit(nc: bass.Bass, adj_blk, s_own, s_all, s_inT, s_outT):
    with tile.TileContext(nc) as tc:
        with (
            # one buffer per j-chunk: no slot reuse -> no extra DMA waits
            tc.tile_pool(name="raw", bufs=JC) as raw_pool,
            tc.tile_pool(name="work", bufs=1) as work,
            tc.tile_pool(name="singles", bufs=1) as singles,
            tc.tile_pool(name="psT", bufs=1, space="PSUM") as psT_pool,
            tc.tile_pool(name="psSin", bufs=1, space="PSUM") as psSin_pool,
            tc.tile_pool(name="psOut", bufs=1, space="PSUM") as psOut_pool,
        ):
            # identity as an inline constant (avoids gpsimd affine_select,
            # whose fill-value register trips Bacc register allocation under
            # Tile reordering); 0/1 are exact in f32r so the bitcast is safe
            ident_dram = nc.inline_tensor(np.eye(P, dtype=np.float32), name="ident_const")
            ident = singles.tile([P, P], F32R)
            nc.sync.dma_start(out=ident, in_=ident_dram.ap().bitcast(F32R))

            # s blocks with i/j on partitions: [p, t, d] = s[t*128 + p, d].
            # Staged through f32 then rounded to f32r (walrus requires fp32r
            # matmul operands to come from fp32r-rounding producers).
            with tc.tile_pool(name="stage", bufs=1) as stage:
                s_own_st = stage.tile([P, IT, D], F32)
                nc.sync.dma_start(
                    out=s_own_st, in_=s_own.rearrange("(t p) d -> p t d", p=P)
                )
                s_own_sb = singles.tile([P, IT, D], F32R)
                nc.vector.tensor_copy(out=s_own_sb, in_=s_own_st)
                s_all_st = stage.tile([P, NJT, D], F32)
                nc.sync.dma_start(
                    out=s_all_st, in_=s_all.rearrange("(t p) d -> p t d", p=P)
                )
                s_all_sb = singles.tile([P, NJT, D], F32R)
                nc.vector.tensor_copy(out=s_all_sb, in_=s_all_st)

            # persistent tiles (no pool slot churn -> minimal waits):
            # reduced-a chunks, double-buffered by chunk parity
            a_chs = [
                [
                    work.tile([P, WJ], F32R, name=f"a_ch_{par}_{it}")
                    for it in range(IT)
                ]
                for par in range(2)
            ]
            # transposed-a SBUF tiles, double-buffered by chunk parity
            aTs = [
                [work.tile([P, RB], F32R, name=f"aT_{par}_{t}") for t in range(JT)]
                for par in range(2)
            ]
            # s_in^T staging, written per chunk, flushed in two half DMAs
            sin_sb_all = work.tile([D, N], F32, name="sin_sb_all")
            # transposed-a PSUM collectors (single set; aT evacuation is fast)
            psT = [
                psT_pool.tile([P, RB], F32R, name=f"psT_{t}") for t in range(JT)
            ]
            # s_in^T chunk accumulators, double-buffered by parity
            psum_sins = [
                psSin_pool.tile([D, WJ], F32, name=f"psum_sin_{par}")
                for par in range(2)
            ]
            # s_out^T accumulator lives across the whole kernel (one bank)
            psum_out = psOut_pool.tile([D, RB], F32)

            # (i_tile, partition) view of the raw block for one-DMA-per-chunk
            adj_r = adj_blk.rearrange("(t p) j k -> p t j k", p=P)

            for jc in range(JC):
                par = jc % 2
                raw = raw_pool.tile([P, IT, WJ, 2], F32, tag="raw")
                nc.sync.dma_start(
                    out=raw, in_=adj_r[:, :, jc * WJ : (jc + 1) * WJ, :]
                )
                psum_sin = psum_sins[par]
                for it in range(IT):
                    a_ch = a_chs[par][it]
                    nc.vector.tensor_add(
                        out=a_ch, in0=raw[:, it, :, 0], in1=raw[:, it, :, 1]
                    )
                    # a^T tiles: psT[t][j, it*128 + i] = a[i, t*128 + j]
                    for t in range(JT):
                        nc.tensor.transpose(
                            psT[t][:, it * P : (it + 1) * P],
                            a_ch[:, t * P : (t + 1) * P],
                            ident,
                        )
                    # s_in^T partial: psum_sin[d, j] += sum_i s_own[i, d] * a[i, j]
                    # (emitted after the transposes: the sin_sb evacuation's
                    # single PE wait then also covers every transpose, so the
                    # aT evacuations below need no PE wait of their own)
                    nc.tensor.matmul(
                        psum_sin,
                        lhsT=s_own_sb[:, it, :],
                        rhs=a_ch,
                        start=(it == 0),
                        stop=(it == IT - 1),
                    )
                # s_in^T partial for this j-chunk is complete -> SBUF -> DRAM.
                # All PSUM evacuations run on DVE so every matmul's upstream
                # dependencies merge into a single DVE semaphore wait (the
                # lowered LDWEIGHTS accepts only one sync wait).
                # DVE evacuation: advances the DVE queue's observed PE clock
                # past this chunk's PE consumers, so later adds reusing a_ch
                # tiles need no PE wait of their own (TT allows only two)
                nc.vector.tensor_copy(
                    out=sin_sb_all[:, jc * WJ : (jc + 1) * WJ], in_=psum_sin
                )
                if jc == JC // 2 - 1:
                    nc.gpsimd.dma_start(
                        out=s_inT[0], in_=sin_sb_all[:, : N // 2]
                    )
                elif jc == JC - 1:
                    nc.gpsimd.dma_start(
                        out=s_inT[1], in_=sin_sb_all[:, N // 2 :]
                    )
                for t in range(JT):
                    jt = jc * JT + t
                    aT = aTs[par][t]
                    nc.scalar.copy(out=aT, in_=psT[t])
                    # s_out^T: psum_out[d, i] += sum_j s[j, d] * a[i, j]
                    nc.tensor.matmul(
                        psum_out,
                        lhsT=s_all_sb[:, jt, :],
                        rhs=aT,
                        start=(jt == 0),
                        stop=(jt == NJT - 1),
                    )
            s_outT_sb = singles.tile([D, RB], F32)
            nc.scalar.copy(out=s_outT_sb, in_=psum_out)
            nc.gpsimd.dma_start(out=s_outT, in_=s_outT_sb)


_ENGINE_SEM_PREFIX = {
    "PE": "PE_",
    "DVE": "DVE_",
    "Activation": "Activation_",
    "Pool": "Pool_",
    "SP": "SP_",
}

_SKIP_OPS = ("InstEventSemaphore", "InstDrain", "InstDMACopy", "InstBranch")


def _strip_self_waits(nc: bass.Bass) -> int:
    """Drop semaphore waits where an instruction waits on its OWN engine's
    completion semaphore.  Engine queues issue and complete in order, so such
    waits are always runtime-satisfied; Tile emits them anyway and they push
    instructions past walrus codegen's per-opcode sync-wait limits (most
    compute encodings accept a single wait)."""
    stripped = 0
    for _, inst in nc.inst_map.items():
        if type(inst).__name__ in _SKIP_OPS:
            continue
        si = getattr(inst, "sync_info", None)
        if si is None or not si.on_wait:
            continue
        eng = getattr(inst, "engine", None)
        prefix = _ENGINE_SEM_PREFIX.get(getattr(eng, "name", ""), None)
        if prefix is None:
            continue
        kept = [w for w in si.on_wait if not w.ant_name.startswith(prefix)]
        if len(kept) != len(si.on_wait):
            stripped += len(si.on_wait) - len(kept)
            si.on_wait = kept
    return stripped


def _build() -> bass.Bass:
    nc = bacc.Bacc("TRN2", num_devices=NCORES)
    adj_blk = nc.dram_tensor("adj_blk", [RB, N, 2], F32, kind="ExternalInput")
    s_own = nc.dram_tensor("s_own", [RB, D], F32, kind="ExternalInput")
    s_all = nc.dram_tensor("s_all", [N, D], F32, kind="ExternalInput")
    # one output tensor per j-chunk so the 8 output DMAs carry no cross-queue
    # write-ordering waits (HWDGE descriptors allow a single sync wait)
    s_inT = [
        nc.dram_tensor(f"s_inT_{h}", [D, N // 2], F32, kind="ExternalOutput")
        for h in range(2)
    ]
    s_outT = nc.dram_tensor("s_outT", [D, RB], F32, kind="ExternalOutput")
    _emit(
        nc,
        adj_blk.ap(),
        s_own.ap(),
        s_all.ap(),
        [t.ap() for t in s_inT],
        s_outT.ap(),
    )
    nc.finalize()
    return nc


_nc_cache = None


def kernel(adj: np.ndarray, s: np.ndarray):
    global _nc_cache, LAST_RESULT
    adj = np.ascontiguousarray(np.asarray(adj, dtype=np.float32))
    s = np.ascontiguousarray(np.asarray(s, dtype=np.float32))
    assert adj.shape == (N, N, 2) and s.shape == (N, D)

    if _nc_cache is None:
        _nc_cache = _build()
    nc = _nc_cache

    in_maps = [
        {
            "adj_blk": np.ascontiguousarray(adj[c * RB : (c + 1) * RB]),
            "s_own": np.ascontiguousarray(s[c * RB : (c + 1) * RB]),
            "s_all": s,
        }
        for c in range(NCORES)
    ]
    res = bass_utils.run_bass_kernel_spmd(
        nc,
        in_maps,
        core_ids=list(range(NCORES)),
        trace=TRACE,
        **TRACE_KWARGS,
    )
    LAST_RESULT = res

    s_in = (
        np.sum(
            [
                np.concatenate([r["s_inT_0"], r["s_inT_1"]], axis=1)
                for r in res.results
            ],
            axis=0,
            dtype=np.float64,
        )
        .astype(np.float32)
        .T
    )
    s_out = np.concatenate([r["s_outT"].T for r in res.results], axis=0)
    return (np.ascontiguousarray(s_in), np.ascontiguousarray(s_out))


# revision 42
# speedup vs baseline: 1.0164x; 1.0164x over previous
"""Trainium2 Bass kernel for nn_CalculateSLayer (GNN message passing).

Computes, for adj (N, N, 2) f32 and s (N, D) f32:
    a     = adj.sum(axis=2)                  # (N, N)
    s_in  = a.T @ s                          # (N, D)
    s_out = a @ s                            # (N, D)
returns (s_in, s_out) — matching the reference's output tuple.

Distribution: adjacency is sharded row-wise across 8 NeuronCores.  Core c
owns rows I_c = [c*512, (c+1)*512).  From its (512, 4096, 2) block it
computes on-device:
  * a partial s_in^T (D, N)   = (s[I_c]).T @ a[I_c]          (contract i)
  * its exact  s_out^T (D,512)= s.T-weighted a[I_c].T        (contract j)
The host then sums the 8 s_in partials and concatenates the s_out blocks.

Per-core dataflow (all engines pipelined under Tile):
  DMA: stream (128, 512j, 2k) f32 chunks of the raw block (4 KB/partition
       contiguous lines — peak-efficiency HBM reads).
  DVE: channel-reduce  a_chunk[:, j] = raw[:, j, 0] + raw[:, j, 1].
  PE : s_in matmul   psum_sin(70,512)  += s_own[it].T @ a_chunk   (f32r moving)
       transposes    psum_T(128,512)[.,it*128:] = a_tile.T        (via identity)
       s_out matmul  psum_out(70,512) += s_all[jt].T @ aT_chunk   (f32r moving)
  ACT: evacuate transposed tiles PSUM->SBUF.
  DMA: psum_sin -> s_inT DRAM per chunk; psum_out -> s_outT DRAM at end.

f32r (float32r) runs the PE at 1 cycle/row for moving free-dim >= 256 (vs 4
cycles/row for plain f32); transposes use the exact permutation path.
"""

import numpy as np

import concourse.bass as bass
from concourse import bacc
import concourse.mybir as mybir
import concourse.tile as tile
from concourse import bass_utils

N = 4096          # nodes
D = 70            # embedding dim
NCORES = 8
RB = N // NCORES  # 512 rows per core
P = 128           # partitions
IT = RB // P      # 4 i-tiles per core
WJ = 512          # j-chunk width
JC = N // WJ      # 8 j-chunks
JT = WJ // P      # 4 transpose subtiles per chunk
NJT = N // P      # 32 s_all subtiles

F32 = mybir.dt.float32
F32R = mybir.dt.float32r

# Set by the test harness to capture a profile; the grading path leaves these
# untouched.
TRACE = False
TRACE_KWARGS = {}
LAST_RESULT = None


def _emit(nc: bass.Bass, adj_blk, s_own, s_all, s_inT, s_outT):
    with tile.TileContext(nc) as tc:
        with (
            # one buffer per (chunk, i-tile): no slot reuse, maximal prefetch
            tc.tile_pool(name="raw", bufs=JC * IT) as raw_pool,
            tc.tile_pool(name="work", bufs=1) as work,
            tc.tile_pool(name="singles", bufs=1) as singles,
            tc.tile_pool(name="psT", bufs=1, space="PSUM") as psT_pool,
            tc.tile_pool(name="psSin", bufs=1, space="PSUM") as psSin_pool,
            tc.tile_pool(name="psOut", bufs=1, space="PSUM") as psOut_pool,
        ):
            # (i_tile, partition) view of the raw block
            adj_r = adj_blk.rearrange("(t p) j k -> p t j k", p=P)

            # issue every raw load up front: per-(chunk, i-tile) granularity
            # so the first adds start as soon as 512 KB lands; the DMA queues
            # then stream the full 16.8 MB back-to-back at HBM rate
            raws = [[None] * IT for _ in range(JC)]
            for jc in range(JC):
                for it in range(IT):
                    r = raw_pool.tile([P, WJ, 2], F32, tag="raw")
                    nc.sync.dma_start(
                        out=r, in_=adj_r[:, it, jc * WJ : (jc + 1) * WJ, :]
                    )
                    raws[jc][it] = r
                if jc == 0:
                    # constants ride the DMA queue right after chunk 0
                    ident_dram = nc.inline_tensor(
                        np.eye(P, dtype=np.float32), name="ident_const"
                    )
                    ident = singles.tile([P, P], F32R)
                    nc.sync.dma_start(
                        out=ident, in_=ident_dram.ap().bitcast(F32R)
                    )
                    with tc.tile_pool(name="stage", bufs=1) as stage:
                        s_own_st = stage.tile([P, IT, D], F32)
                        nc.sync.dma_start(
                            out=s_own_st,
                            in_=s_own.rearrange("(t p) d -> p t d", p=P),
                        )
                        s_own_sb = singles.tile([P, IT, D], F32R)
                        nc.vector.tensor_copy(out=s_own_sb, in_=s_own_st)
                        s_all_st = stage.tile([P, NJT, D], F32)
                        nc.sync.dma_start(
                            out=s_all_st,
                            in_=s_all.rearrange("(t p) d -> p t d", p=P),
                        )
                        s_all_sb = singles.tile([P, NJT, D], F32R)
                        nc.vector.tensor_copy(out=s_all_sb, in_=s_all_st)

            # persistent working tiles
            a_chs = [
                [
                    work.tile([P, WJ], F32R, name=f"a_ch_{par}_{it}")
                    for it in range(IT)
                ]
                for par in range(2)
            ]
            aTs = [
                [work.tile([P, RB], F32R, name=f"aT_{par}_{t}") for t in range(JT)]
                for par in range(2)
            ]
            sin_sb_all = work.tile([D, N], F32, name="sin_sb_all")
            psT = [
                psT_pool.tile([P, RB], F32R, name=f"psT_{t}") for t in range(JT)
            ]
            psum_sins = [
                psSin_pool.tile([D, WJ], F32, name=f"psum_sin_{par}")
                for par in range(2)
            ]
            psum_out = psOut_pool.tile([D, RB], F32)

            def emit_sout_mms(jc):
                """s_out^T += s_all[jt].T @ aT for chunk jc (aTs already
                evacuated; runs one chunk behind so the PE never stalls on
                the PSUM->SBUF copies)."""
                par = jc % 2
                for t in range(JT):
                    jt = jc * JT + t
                    nc.tensor.matmul(
                        psum_out,
                        lhsT=s_all_sb[:, jt, :],
                        rhs=aTs[par][t],
                        start=(jt == 0),
                        stop=(jt == NJT - 1),
                    )

            for jc in range(JC):
                par = jc % 2
                psum_sin = psum_sins[par]
                for it in range(IT):
                    raw = raws[jc][it]
                    a_ch = a_chs[par][it]
                    nc.vector.tensor_add(
                        out=a_ch, in0=raw[:, :, 0], in1=raw[:, :, 1]
                    )
                    # a^T tiles: psT[t][j, it*128 + i] = a[i, t*128 + j]
                    for t in range(JT):
                        nc.tensor.transpose(
                            psT[t][:, it * P : (it + 1) * P],
                            a_ch[:, t * P : (t + 1) * P],
                            ident,
                        )
                    # s_in^T partial: psum_sin[d, j] += sum_i s_own[i, d]*a[i, j]
                    nc.tensor.matmul(
                        psum_sin,
                        lhsT=s_own_sb[:, it, :],
                        rhs=a_ch,
                        start=(it == 0),
                        stop=(it == IT - 1),
                    )
                    if it == IT - 1 and jc > 0:
                        # previous chunk's s_out matmuls: their aT operands
                        # finished copying while this chunk transposed
                        emit_sout_mms(jc - 1)
                # evacuate s_in^T chunk (DVE) and a^T tiles (ACT), then flush
                nc.vector.tensor_copy(
                    out=sin_sb_all[:, jc * WJ : (jc + 1) * WJ], in_=psum_sin
                )
                for t in range(JT):
                    nc.scalar.copy(out=aTs[par][t], in_=psT[t])
                if jc == JC // 2 - 1:
                    nc.gpsimd.dma_start(out=s_inT[0], in_=sin_sb_all[:, : N // 2])
                elif jc == JC - 1:
                    nc.gpsimd.dma_start(out=s_inT[1], in_=sin_sb_all[:, N // 2 :])
            emit_sout_mms(JC - 1)
            s_outT_sb = singles.tile([D, RB], F32)
            nc.scalar.copy(out=s_outT_sb, in_=psum_out)
            nc.gpsimd.dma_start(out=s_outT, in_=s_outT_sb)


_ENGINE_SEM_PREFIX = {
    "PE": "PE_",
    "DVE": "DVE_",
    "Activation": "Activation_",
    "Pool": "Pool_",
    "SP": "SP_",
}

_SKIP_OPS = ("InstEventSemaphore", "InstDrain", "InstDMACopy", "InstBranch")


def _strip_self_waits(nc: bass.Bass) -> int:
    """Drop semaphore waits where an instruction waits on its OWN engine's
    completion semaphore.  Engine queues issue and complete in order, so such
    waits are always runtime-satisfied; Tile emits them anyway and they push
    instructions past walrus codegen's per-opcode sync-wait limits (most
    compute encodings accept a single wait)."""
    stripped = 0
    for _, inst in nc.inst_map.items():
        if type(inst).__name__ in _SKIP_OPS:
            continue
        si = getattr(inst, "sync_info", None)
        if si is None or not si.on_wait:
            continue
        eng = getattr(inst, "engine", None)
        prefix = _ENGINE_SEM_PREFIX.get(getattr(eng, "name", ""), None)
        if prefix is None:
            continue
        kept = [w for w in si.on_wait if not w.ant_name.startswith(prefix)]
        if len(kept) != len(si.on_wait):
            stripped += len(si.on_wait) - len(kept)
            si.on_wait = kept
    return stripped


def _build() -> bass.Bass:
    nc = bacc.Bacc("TRN2", num_devices=NCORES)
    adj_blk = nc.dram_tensor("adj_blk", [RB, N, 2], F32, kind="ExternalInput")
    s_own = nc.dram_tensor("s_own", [RB, D], F32, kind="ExternalInput")
    s_all = nc.dram_tensor("s_all", [N, D], F32, kind="ExternalInput")
    # one output tensor per j-chunk so the 8 output DMAs carry no cross-queue
    # write-ordering waits (HWDGE descriptors allow a single sync wait)
    s_inT = [
        nc.dram_tensor(f"s_inT_{h}", [D, N // 2], F32, kind="ExternalOutput")
        for h in range(2)
    ]
    s_outT = nc.dram_tensor("s_outT", [D, RB], F32, kind="ExternalOutput")
    _emit(
        nc,
        adj_blk.ap(),
        s_own.ap(),
        s_all.ap(),
        [t.ap() for t in s_inT],
        s_outT.ap(),
    )
    nc.finalize()
    return nc


_nc_cache = None


def kernel(adj: np.ndarray, s: np.ndarray):
    global _nc_cache, LAST_RESULT
    adj = np.ascontiguousarray(np.asarray(adj, dtype=np.float32))
    s = np.ascontiguousarray(np.asarray(s, dtype=np.float32))
    assert adj.shape == (N, N, 2) and s.shape == (N, D)

    if _nc_cache is None:
        _nc_cache = _build()
    nc = _nc_cache

    in_maps = [
        {
            "adj_blk": np.ascontiguousarray(adj[c * RB : (c + 1) * RB]),
            "s_own": np.ascontiguousarray(s[c * RB : (c + 1) * RB]),
            "s_all": s,
        }
        for c in range(NCORES)
    ]
    res = bass_utils.run_bass_kernel_spmd(
        nc,
        in_maps,
        core_ids=list(range(NCORES)),
        trace=TRACE,
        **TRACE_KWARGS,
    )
    LAST_RESULT = res

    s_in = (
        np.sum(
            [
                np.concatenate([r["s_inT_0"], r["s_inT_1"]], axis=1)
                for r in res.results
            ],
            axis=0,
            dtype=np.float64,
        )
        .astype(np.float32)
        .T
    )
    s_out = np.concatenate([r["s_outT"].T for r in res.results], axis=0)
    return (np.ascontiguousarray(s_in), np.ascontiguousarray(s_out))
